# revision 1
# baseline (speedup 1.0000x reference)
"""Causal attention kernel for Trainium2 (Bass/Tile), 8-core SPMD.

Problem: x:(4,2048,1024), w_{q,k,v}:(1024,1024) fp32.
  q/k/v = x @ w.T ; scores = (q @ k.T)/sqrt(1024) causal-masked; out = softmax @ v.

Sharding: core c -> batch b=c//2, half h=c%2. Each batch's 16 query blocks
(128 rows) are interleaved even/odd between its two cores (core-local block
j <-> global block g=2j+h), so causal work is balanced. Every core computes
K^T and V for the whole batch (keys), Q^T only for its own 1024 query rows.
The program is identical on all cores (SPMD); the only per-core difference is
input DATA: which rows go into xqT, and a small additive causal mask tile
whose diagonal offset encodes h.

Layouts (host pre-transposes, so no on-chip transposes of inputs needed):
  xkvT  [D, T]  = x[b].T          (d_in on partitions for projections)
  xqT   [D, 1024] = x[b][qrows].T
  w*T   [D, D]  = w.T             ([d_in, d_out])
  cmask [128, 256] additive mask for the last two key blocks of each qblock
Kernel keeps K^T [o,t], V [t,o], Q^T [o,t] in SBUF (projections computed
with dc-outer loops accumulating 8 PSUM banks in parallel, weights fully
resident, DMA prologues interleaved so compute starts after ~768KB), then
per query block (largest first): scores into PSUM slices (PE) -> mask add
(DVE) -> exp+row-sum straight from PSUM (ACT, fused accum_out) -> P^T via
PE transpose -> context matmul (PE) -> scale by 1/rowsum (DVE).
Matmul datapath is float32r: fp32 storage, single-pass PE multiply
(1 cyc/row at N>=256, 4x the fp32 rate) with fp32 PSUM accumulation;
measured output rel err vs fp32 reference ~3e-4. Set _MM_MODE="fp32" for
bit-conservative (4x slower) matmuls.
"""

import numpy as np

_B, _T, _D = 4, 2048, 1024
_P = 128
_NQB = 8              # query blocks per core
_TQ = _NQB * _P       # 1024 query rows per core
_GAMMA = 1.0 / 32.0   # 1/sqrt(D)
_NEG = -1.0e9

# matmul input precision: "fp32" (exact, 4 cyc/row), "fp32r" (1 cyc/row at
# N>=256, ~tf32 multiply precision), "bf16" (1 cyc/row, inputs rounded)
_MM_MODE = "fp32r"
# softmax max-subtraction: scores are ~N(0,1) after scaling, exp() cannot
# overflow, and softmax is shift-invariant — skipping the row-max pass
# removes a DVE reduce + serialization before the ACT exp.
_SUB_MAX = False

_REPLICA_GROUPS = [[0, 1], [2, 3], [4, 5], [6, 7]]
_CACHE = {}


def _build_nc(mm_mode: str, sub_max: bool = True):
    import concourse.bass as bass  # noqa: F401
    import concourse.mybir as mybir
    import concourse.tile as tile
    from concourse import bacc
    from concourse.masks import make_identity
    from contextlib import ExitStack

    f32 = mybir.dt.float32
    if mm_mode == "bf16":
        mdt = mybir.dt.bfloat16
    elif mm_mode == "fp32r":
        mdt = mybir.dt.float32r
    else:
        mdt = f32

    def mm(x):
        return x

    nc = bacc.Bacc(None, target_bir_lowering=False)
    # xkvT now holds only this core's OWN key half (keys [1024h, 1024h+1024))
    xkvT = nc.dram_tensor("xkvT", [_D, _T // 2], mdt, kind="ExternalInput")
    xqT = nc.dram_tensor("xqT", [_D, _TQ], mdt, kind="ExternalInput")
    wqT = nc.dram_tensor("wqT", [_D, _D], mdt, kind="ExternalInput")
    wkT = nc.dram_tensor("wkT", [_D, _D], mdt, kind="ExternalInput")
    wvT = nc.dram_tensor("wvT", [_D, _D], mdt, kind="ExternalInput")
    cmask = nc.dram_tensor("cmask", [_P, 2 * _P], f32, kind="ExternalInput")
    out = nc.dram_tensor("out", [_TQ, _D], f32, kind="ExternalOutput")

    xkvT_v = xkvT.rearrange("(a p) t -> p a t", p=_P)   # [128, 8, 1024]
    xqT_v = xqT.rearrange("(a p) t -> p a t", p=_P)     # [128, 8, 1024]
    wqT_v = wqT.rearrange("(a p) o -> p a o", p=_P)
    wkT_v = wkT.rearrange("(a p) o -> p a o", p=_P)
    wvT_v = wvT.rearrange("(a p) o -> p a o", p=_P)

    with ExitStack() as ctx:
        tc = ctx.enter_context(tile.TileContext(nc))
        const = ctx.enter_context(tc.tile_pool(name="const", bufs=1))

        if mdt == f32:
            ident = const.tile([_P, _P], f32, tag="ident")
            make_identity(nc, ident)
        else:
            ident_f32 = const.tile([_P, _P], f32, tag="identf")
            make_identity(nc, ident_f32)
            ident = const.tile([_P, _P], mdt, tag="ident")
            nc.vector.tensor_copy(ident, ident_f32)
        cmask_sb = const.tile([_P, 2 * _P], f32, tag="cmask")
        nc.sync.dma_start(out=cmask_sb, in_=cmask[:, :])

        # DRAM bounce buffers for the pair AllGather (K^T half + V half).
        dramp = ctx.enter_context(tc.tile_pool(name="dram", bufs=1,
                                               space="DRAM"))
        in_bk = dramp.tile([_T // 2, _D], mdt, tag="inbk")
        out_bk = dramp.tile([_T, _D], mdt, tag="outbk")
        in_bv = dramp.tile([_T // 2, _D], mdt, tag="inbv")
        out_bv = dramp.tile([_T, _D], mdt, tag="outbv")

        # ---- Phase A: K^T-half and V-half from own xkvT (fully resident) ----
        with tc.tile_pool(name="ph", bufs=1) as ph, \
             tc.tile_pool(name="pw", bufs=1) as pw:
            KTh = ph.tile([_P, 8, _T // 2], mdt, tag="kth")
            Vh = ph.tile([_P, 8, _D], mdt, tag="vh")
            wk_sb = pw.tile([_P, 8, _D], mdt, name="wk_sb", tag="wk")
            wv_sb = pw.tile([_P, 8, _D], mdt, name="wv_sb", tag="wv")
            xh = pw.tile([_P, 8, _T // 2], mdt, name="xh", tag="xh")
            for dc in range(8):
                nc.sync.dma_start(out=wk_sb[:, dc, :], in_=wkT_v[:, dc, :])
                nc.sync.dma_start(out=xh[:, dc, :], in_=xkvT_v[:, dc, :])
            for dc in range(8):
                nc.sync.dma_start(out=wv_sb[:, dc, :], in_=wvT_v[:, dc, :])

            with tc.tile_pool(name="ps_k", bufs=1, space="PSUM") as pp:
                for ts in range(2):
                    ps = [pp.tile([_P, 512], f32, name=f"ps{oc}",
                                  tag=f"s{oc}") for oc in range(8)]
                    for dc in range(8):
                        for oc in range(8):
                            nc.tensor.matmul(
                                ps[oc], mm(wk_sb[:, dc, oc * _P:(oc + 1) * _P]),
                                mm(xh[:, dc, ts * 512:(ts + 1) * 512]),
                                start=(dc == 0), stop=(dc == 7))
                    for oc in range(8):
                        nc.scalar.copy(KTh[:, oc, ts * 512:(ts + 1) * 512],
                                       ps[oc])
            for oc in range(8):
                nc.sync.dma_start(out=in_bk[oc * _P:(oc + 1) * _P, :],
                                  in_=KTh[:, oc, :])
            nc.gpsimd.collective_compute(
                "AllGather", mybir.AluOpType.bypass,
                replica_groups=_REPLICA_GROUPS,
                ins=[in_bk.opt()], outs=[out_bk.opt()])

            with tc.tile_pool(name="ps_v", bufs=1, space="PSUM") as pp:
                for ts in range(2):
                    ps = [pp.tile([_P, _D], f32, name=f"psv{tt}",
                                  tag=f"v{tt}") for tt in range(4)]
                    for dc in range(8):
                        for tt in range(4):
                            for ns in range(2):
                                nc.tensor.matmul(
                                    ps[tt][:, ns * 512:(ns + 1) * 512],
                                    mm(xh[:, dc,
                                          ts * 512 + tt * _P:
                                          ts * 512 + (tt + 1) * _P]),
                                    mm(wv_sb[:, dc, ns * 512:(ns + 1) * 512]),
                                    start=(dc == 0), stop=(dc == 7))
                    for tt in range(4):
                        nc.scalar.copy(Vh[:, ts * 4 + tt, :], ps[tt])

            for tt in range(8):
                nc.sync.dma_start(out=in_bv[tt * _P:(tt + 1) * _P, :],
                                  in_=Vh[:, tt, :])

        # V-half AllGather (K-half gather already in flight)
        nc.gpsimd.collective_compute(
            "AllGather", mybir.AluOpType.bypass,
            replica_groups=_REPLICA_GROUPS,
            ins=[in_bv.opt()], outs=[out_bv.opt()])

        # ---- Phase B: Q^T (full), overlaps the collective ----
        pqt = ctx.enter_context(tc.tile_pool(name="pqt", bufs=1))
        QT = pqt.tile([_P, 8, _TQ], mdt, tag="qt")
        with tc.tile_pool(name="pb_w", bufs=1) as pw, \
             tc.tile_pool(name="pb_x", bufs=12) as px, \
             tc.tile_pool(name="pb_ps", bufs=1, space="PSUM") as pp:
            wq_sb = pw.tile([_P, 8, _D], mdt, tag="w")
            xt0 = []
            for dc in range(8):
                nc.sync.dma_start(out=wq_sb[:, dc, :], in_=wqT_v[:, dc, :])
                xt = px.tile([_P, 512], mdt, name=f"xb0_{dc}", tag="xs")
                nc.sync.dma_start(out=xt, in_=xqT_v[:, dc, 0:512])
                xt0.append(xt)
            for ts in range(2):
                ps = [pp.tile([_P, 512], f32, name=f"ps{oc}", tag=f"s{oc}")
                      for oc in range(8)]
                for dc in range(8):
                    if ts == 0:
                        xt = xt0[dc]
                    else:
                        xt = px.tile([_P, 512], mdt, tag="xs")
                        nc.sync.dma_start(
                            out=xt, in_=xqT_v[:, dc, ts * 512:(ts + 1) * 512])
                    for oc in range(8):
                        nc.tensor.matmul(
                            ps[oc], mm(wq_sb[:, dc, oc * _P:(oc + 1) * _P]),
                            mm(xt), start=(dc == 0), stop=(dc == 7))
                for oc in range(8):
                    nc.scalar.copy(QT[:, oc, ts * 512:(ts + 1) * 512], ps[oc])

        # ---- readback of gathered K^T/V, key-order = [rank0, rank1] ----
        pkv = ctx.enter_context(tc.tile_pool(name="pkv", bufs=1))
        KTg = [pkv.tile([_P, 8, _T // 2], mdt, name=f"kt{hh}", tag=f"kt{hh}")
               for hh in range(2)]
        Vg = [pkv.tile([_P, 8, _D], mdt, name=f"v{hh}", tag=f"v{hh}")
              for hh in range(2)]
        for hh in range(2):
            for oc in range(8):
                nc.sync.dma_start(
                    out=KTg[hh][:, oc, :],
                    in_=out_bk[(_T // 2) * hh + oc * _P:
                               (_T // 2) * hh + (oc + 1) * _P, :])
        for hh in range(2):
            for tt in range(8):
                nc.sync.dma_start(
                    out=Vg[hh][:, tt, :],
                    in_=out_bv[(_T // 2) * hh + tt * _P:
                               (_T // 2) * hh + (tt + 1) * _P, :])

        # ---------------- Phase C: attention per query block ----------------
        with tc.tile_pool(name="pc_p", bufs=2) as ppsb, \
             tc.tile_pool(name="pc_pt", bufs=3) as ppt, \
             tc.tile_pool(name="pc_ctx", bufs=2) as pctx, \
             tc.tile_pool(name="pc_small", bufs=4) as psm, \
             tc.tile_pool(name="pc_ps_s", bufs=2, space="PSUM") as pps, \
             tc.tile_pool(name="pc_ps_t", bufs=2, space="PSUM") as ppts, \
             tc.tile_pool(name="pc_ps_c", bufs=2, space="PSUM") as ppc:
            for j in reversed(range(_NQB)):
                km = 256 * (j + 1)
                nkb = 2 * (j + 1)
                nsl = (km + 511) // 512
                pexp = ppsb.tile([_P, _T], mdt, tag="pexp")
                denoms = psm.tile([_P, 4], f32, tag="denoms")
                for ks in range(nsl):
                    w = min(512, km - ks * 512)
                    ps = pps.tile([_P, 512], f32, tag="s")
                    kth = KTg[ks // 2]
                    kcol = (ks % 2) * 512
                    for oc in range(8):
                        nc.tensor.matmul(
                            ps[:, :w],
                            mm(QT[:, oc, j * _P:(j + 1) * _P]),
                            mm(kth[:, oc, kcol:kcol + w]),
                            start=(oc == 0), stop=(oc == 7))
                    if ks == nsl - 1:
                        nc.vector.tensor_add(
                            ps[:, w - 256:w], ps[:, w - 256:w], cmask_sb)
                    nc.scalar.activation(
                        out=pexp[:, ks * 512:ks * 512 + w], in_=ps[:, :w],
                        func=mybir.ActivationFunctionType.Exp,
                        bias=0.0, scale=_GAMMA,
                        accum_out=denoms[:, ks:ks + 1])

                denom = psm.tile([_P, 1], f32, tag="denom")
                nc.vector.tensor_reduce(
                    out=denom, in_=denoms[:, :nsl],
                    axis=mybir.AxisListType.X, op=mybir.AluOpType.add)
                rden = psm.tile([_P, 1], f32, tag="rden")
                nc.vector.reciprocal(rden, denom)

                ctx_ps = ppc.tile([_P, _D], f32, tag="ctx")
                for kb in range(nkb):
                    pt_ps = ppts.tile([_P, _P], mdt, tag="pt")
                    nc.tensor.transpose(
                        pt_ps, pexp[:, kb * _P:(kb + 1) * _P], ident)
                    pt_sb = ppt.tile([_P, _P], mdt, tag="pts")
                    nc.vector.tensor_copy(pt_sb, pt_ps)
                    vsrc = Vg[kb // 8][:, kb % 8, :]
                    for ns in range(2):
                        nc.tensor.matmul(
                            ctx_ps[:, ns * 512:(ns + 1) * 512],
                            mm(pt_sb),
                            mm(vsrc[:, ns * 512:(ns + 1) * 512]),
                            start=(kb == 0), stop=(kb == nkb - 1))
                ctx_sb = pctx.tile([_P, _D], f32, tag="ctxsb")
                nc.vector.tensor_scalar_mul(ctx_sb, ctx_ps, rden)
                nc.sync.dma_start(
                    out=out[j * _P:(j + 1) * _P, :], in_=ctx_sb)

    nc.finalize()
    return nc


def _qrows(h: int) -> np.ndarray:
    """Global query-row indices handled by half h, in core-local order."""
    blocks = np.arange(_NQB) * 2 + h          # global block ids, 8 of them
    return (blocks[:, None] * _P + np.arange(_P)[None, :]).reshape(-1)


def _host_inputs(x, w_query, w_key, w_value, mm_mode: str):
    if mm_mode == "bf16":
        import ml_dtypes
        cdt = ml_dtypes.bfloat16
    else:
        cdt = np.float32
    wqT = np.ascontiguousarray(np.asarray(w_query, np.float32).T).astype(cdt)
    wkT = np.ascontiguousarray(np.asarray(w_key, np.float32).T).astype(cdt)
    wvT = np.ascontiguousarray(np.asarray(w_value, np.float32).T).astype(cdt)
    x = np.asarray(x, np.float32)

    in_maps = []
    for c in range(8):
        b, h = c // 2, c % 2
        xb = x[b]                                     # [T, D]
        qr = _qrows(h)
        xkvT = np.ascontiguousarray(xb[1024 * h:1024 * (h + 1)].T).astype(cdt)
        xqT = np.ascontiguousarray(xb[qr].T).astype(cdt)        # [D, 1024]
        # cmask[p, c2] = 0 if c2 <= p + 128*h else -1e9
        p = np.arange(_P)[:, None]
        c2 = np.arange(2 * _P)[None, :]
        cmask = np.where(c2 <= p + _P * h, 0.0, _NEG).astype(np.float32)
        in_maps.append({
            "xkvT": xkvT, "xqT": xqT,
            "wqT": wqT, "wkT": wkT, "wvT": wvT,
            "cmask": cmask,
        })
    return in_maps


def _gather(results):
    out = np.empty((_B, _T, _D), np.float32)
    for c in range(8):
        b, h = c // 2, c % 2
        out[b, _qrows(h)] = results[c]["out"]
    return out


def kernel(x, w_query, w_key, w_value, _trace=False):
    key = (_MM_MODE, _SUB_MAX)
    if key not in _CACHE:
        _CACHE[key] = _build_nc(_MM_MODE, _SUB_MAX)
    nc = _CACHE[key]
    in_maps = _host_inputs(x, w_query, w_key, w_value, _MM_MODE)
    from concourse.bass_utils import run_bass_kernel_spmd
    res = run_bass_kernel_spmd(nc, in_maps, core_ids=list(range(8)),
                               trace=_trace)
    out = _gather(res.results)
    if _trace:
        return out, res
    return out



# revision 18
# speedup vs baseline: 1.0642x; 1.0642x over previous
"""Causal attention kernel for Trainium2 (Bass/Tile), 8-core SPMD.

Problem: x:(4,2048,1024), w_{q,k,v}:(1024,1024) fp32.
  q/k/v = x @ w.T ; scores = (q @ k.T)/sqrt(1024) causal-masked; out = softmax @ v.

Sharding: core c -> batch b=c//2, half h=c%2. Each batch's 16 query blocks
(128 rows) are interleaved even/odd between its two cores (core-local block
j <-> global block g=2j+h), so causal work is balanced. Q^T is projected for
the core's own 1024 query rows only; K^T and V are projected IN FULL on both
cores of a pair (redundant compute, ~55us extra PE time) — this removes the
pair AllGather of the earlier design, which serialized ~450us of collective
time and stalled every engine. Zero inter-core communication remains; the
program is SPMD (per-core difference is input DATA: xqT row choice and the
cmask diagonal offset encoding h).

Layouts (host pre-transposes, so no on-chip transposes of inputs needed):
  xT    [D, T]    = x[b].T        (d_in on partitions for K/V projections)
  xqT   [D, 1024] = x[b][qrows].T
  w*T   [D, D]    = w.T           ([d_in, d_out])
  cmask [128, 256] additive mask for the last two key blocks of each qblock
SBUF residency (per partition): KT fp32r 64KB + V bf16 32KB + QT fp32r 32KB;
V (and the softmax weights P) are bf16 so full K/V fit without collectives.
Phase A streams x in four [128,8,512] chunks (double-buffered): per chunk
K^T cols (dc-outer, 8 PSUM banks) then V rows (2-bank tiles, tb-serial),
PSUM evacuations alternate ACT/DVE so the PE never waits long on a single
evac chain. Phase B projects Q^T (ts=1 first so attention can start early).
Phase C per query block (largest first): scores into PSUM slices (PE) ->
mask add (DVE) -> exp+row-sum straight from PSUM (ACT, fused accum_out,
bf16 out) -> P^T via PE transpose -> context matmul (PE, bf16) -> scale by
1/rowsum (DVE) -> DMA out.
Matmul datapath: fp32r for projections/scores (fp32 storage, 1 cyc/row at
N>=256), bf16 for P@V. Measured output rel err vs fp32 reference ~6e-4.
"""

import numpy as np

_B, _T, _D = 4, 2048, 1024
_P = 128
_NQB = 8              # query blocks per core
_TQ = _NQB * _P       # 1024 query rows per core
_GAMMA = 1.0 / 32.0   # 1/sqrt(D)
_NEG = -1.0e9

_MM_MODE = "fp32r"    # projections/scores datapath
_SUB_MAX = False      # scores ~N(0,1): exp cannot overflow, skip row-max

_CACHE = {}


def _build_nc(mm_mode: str, sub_max: bool = False):
    import concourse.bass as bass  # noqa: F401
    import concourse.mybir as mybir
    import concourse.tile as tile
    from concourse import bacc
    from concourse.masks import make_identity
    from contextlib import ExitStack

    f32 = mybir.dt.float32
    bf16 = mybir.dt.bfloat16
    if mm_mode == "bf16":
        mdt = bf16
    elif mm_mode == "fp32r":
        mdt = mybir.dt.float32r
    else:
        mdt = f32

    nc = bacc.Bacc(None, target_bir_lowering=False)
    xT = nc.dram_tensor("xT", [_D, _T], mdt, kind="ExternalInput")
    xqT = nc.dram_tensor("xqT", [_D, _TQ], mdt, kind="ExternalInput")
    wqT = nc.dram_tensor("wqT", [_D, _D], mdt, kind="ExternalInput")
    wkT = nc.dram_tensor("wkT", [_D, _D], mdt, kind="ExternalInput")
    wvT = nc.dram_tensor("wvT", [_D, _D], mdt, kind="ExternalInput")
    cmask = nc.dram_tensor("cmask", [_P, 2 * _P], f32, kind="ExternalInput")
    out = nc.dram_tensor("out", [_TQ, _D], f32, kind="ExternalOutput")

    xT_v = xT.rearrange("(a p) t -> p a t", p=_P)       # [128, 8, 2048]
    xqT_v = xqT.rearrange("(a p) t -> p a t", p=_P)     # [128, 8, 1024]
    wqT_v = wqT.rearrange("(a p) o -> p a o", p=_P)
    wkT_v = wkT.rearrange("(a p) o -> p a o", p=_P)
    wvT_v = wvT.rearrange("(a p) o -> p a o", p=_P)

    _NCH = 4            # x chunks of 512 key-cols
    _CW = _T // _NCH    # 512

    with ExitStack() as ctx:
        tc = ctx.enter_context(tile.TileContext(nc))
        const = ctx.enter_context(tc.tile_pool(name="const", bufs=1))

        ident_f32 = const.tile([_P, _P], f32, tag="identf")
        make_identity(nc, ident_f32)
        identb = const.tile([_P, _P], bf16, tag="identb")
        nc.vector.tensor_copy(identb, ident_f32)
        cmask_sb = const.tile([_P, 2 * _P], f32, tag="cmask")
        nc.sync.dma_start(out=cmask_sb, in_=cmask[:, :])

        # persistent SBUF residents (QT allocated after phase A pools close)
        pres = ctx.enter_context(tc.tile_pool(name="pres", bufs=1))
        KT = pres.tile([_P, 8, _T], mdt, tag="kt")          # [o-part, oc, t]
        V = pres.tile([_P, 16, _D], bf16, tag="v")          # [t-part, tt, o]
        # pre-allocated staging tile: dodges the pool-release drain barrier
        # so the first xq quarter can land during phase A
        pst = ctx.enter_context(tc.tile_pool(name="pst", bufs=1))
        xq3_st = pst.tile([_P, 8, 256], mdt, tag="st")

        # ---- Phase A: K-pass then V-pass. x is streamed twice so wk+wv
        # are the only resident weights; DMA queue order matches
        # consumption order so the PE never waits mid-pass. ----
        with tc.tile_pool(name="pa_x", bufs=2) as px, \
             tc.tile_pool(name="pa_w", bufs=1) as pw, \
             tc.tile_pool(name="pa_ps", bufs=1, space="PSUM") as pp:

            def load_chunk(chi, nm, eng=None):
                ch = px.tile([_P, 8, _CW], mdt, name=nm, tag="xc")
                (eng or nc.sync).dma_start(
                    out=ch, in_=xT_v[:, :, chi * _CW:(chi + 1) * _CW])
                return ch

            wk_sb = pw.tile([_P, 8, _D], mdt, tag="wk")
            wv_sb = pw.tile([_P, 8, _D], mdt, tag="wv")
            # SP queue: wk/ch0 interleaved, ch1, ch2, wv, ch3, xq3 stage,
            # then (emitted later) wq, xq, out. V-pass x reloads go on the
            # idle Pool queue so phase B loads aren't stuck behind them.
            ch0 = px.tile([_P, 8, _CW], mdt, name="ch0", tag="xc")
            for dc in range(8):
                nc.sync.dma_start(out=wk_sb[:, dc, :], in_=wkT_v[:, dc, :])
                nc.sync.dma_start(out=ch0[:, dc, :], in_=xT_v[:, dc, 0:_CW])
            ch1 = load_chunk(1, "ch1")
            ch2 = load_chunk(2, "ch2")
            for dc in range(8):
                nc.sync.dma_start(out=wv_sb[:, dc, :], in_=wvT_v[:, dc, :])
            ch3 = load_chunk(3, "ch3")
            nc.sync.dma_start(out=xq3_st,
                              in_=xqT_v[:, :, 3 * 256:4 * 256])

            # K-pass. ch0 is dc-outer (matches the interleaved DMA arrival
            # order); ch1-3 are oc-outer so each bank's evacuation pipelines
            # under the next oc's accumulation instead of bunching at the
            # chunk boundary (which would stall the PE and reset its p-state
            # ramp to 1.2GHz).
            def k_evac(chi, oc, t):
                dst = KT[:, oc, chi * _CW:(chi + 1) * _CW]
                if oc % 2 == 0:
                    nc.scalar.copy(dst, t)
                else:
                    nc.vector.tensor_copy(dst, t)

            ps = [pp.tile([_P, _CW], f32, name=f"k0_{oc}", tag=f"b{oc}")
                  for oc in range(8)]
            for dc in range(8):
                for oc in range(8):
                    nc.tensor.matmul(
                        ps[oc], wk_sb[:, dc, oc * _P:(oc + 1) * _P],
                        ch0[:, dc, :], start=(dc == 0), stop=(dc == 7))
            for oc in range(8):
                k_evac(0, oc, ps[oc])
            for chi, ch in ((1, ch1), (2, ch2), (3, ch3)):
                for oc in range(8):
                    t = pp.tile([_P, _CW], f32, name=f"k{chi}_{oc}",
                                tag=f"b{oc}")
                    for dc in range(8):
                        nc.tensor.matmul(
                            t, wk_sb[:, dc, oc * _P:(oc + 1) * _P],
                            ch[:, dc, :], start=(dc == 0), stop=(dc == 7))
                    k_evac(chi, oc, t)

            # V-pass (x streamed again, reloads on the Pool DMA queue)
            for chi in range(4):
                ch = load_chunk(chi, f"cv{chi}", eng=nc.gpsimd)
                for tb in range(4):
                    psv = [pp.tile([_P, 512], f32, name=f"v{chi}_{tb}_{ns}",
                                   tag=f"b{2 * tb + ns}") for ns in range(2)]
                    for dc in range(8):
                        for ns in range(2):
                            nc.tensor.matmul(
                                psv[ns],
                                ch[:, dc, tb * _P:(tb + 1) * _P],
                                wv_sb[:, dc, ns * 512:(ns + 1) * 512],
                                start=(dc == 0), stop=(dc == 7))
                    tt = chi * 4 + tb
                    nc.scalar.copy(V[:, tt, 0:512], psv[0])
                    nc.vector.tensor_copy(V[:, tt, 512:_D], psv[1])

        # ---- Phases B+C share one scope: QT lives through attention;
        # one 8-bank PSUM pool serves Q projection (tags b0..b7), then
        # scores (b0/b1), P^T (b2/b3) and context halves (b4..b7). ----
        with tc.tile_pool(name="pb_qt", bufs=1) as pqt, \
             tc.tile_pool(name="pb_w", bufs=1) as pwq, \
             tc.tile_pool(name="pb_x", bufs=2) as pxq, \
             tc.tile_pool(name="pc_p", bufs=2) as ppsb, \
             tc.tile_pool(name="pc_pt", bufs=3) as ppt, \
             tc.tile_pool(name="pc_ctx", bufs=2) as pctx, \
             tc.tile_pool(name="pc_small", bufs=4) as psm, \
             tc.tile_pool(name="pbc_ps", bufs=1, space="PSUM") as pb:
            QT = pqt.tile([_P, 8, _TQ], mdt, tag="qt")      # [o-part, oc, q]
            # wq streamed as four dc-pair quarters [*,2,1024] so the Q
            # matmuls start as soon as the first quarter lands (the pool
            # alloc itself is gated on phase A's drain). xq quarter tsq=3
            # was pre-staged during phase A.
            wq_q = [pwq.tile([_P, 2, _D], mdt, name=f"wqq{qq}",
                             tag=f"wq{qq}") for qq in range(4)]
            xq_t = {3: xq3_st}
            for qq in (0, 1):
                nc.sync.dma_start(out=wq_q[qq],
                                  in_=wqT_v[:, 2 * qq:2 * qq + 2, :])
            for tsq in (2, 1, 0):
                xq_t[tsq] = pxq.tile([_P, 8, 256], mdt, name=f"xq{tsq}",
                                     tag="xq")
                if tsq == 2:
                    nc.sync.dma_start(
                        out=xq_t[2], in_=xqT_v[:, :, 2 * 256:3 * 256])
                    for qq in (2, 3):
                        nc.sync.dma_start(out=wq_q[qq],
                                          in_=wqT_v[:, 2 * qq:2 * qq + 2, :])
                else:
                    nc.sync.dma_start(
                        out=xq_t[tsq],
                        in_=xqT_v[:, :, tsq * 256:(tsq + 1) * 256])
            # ts=1 first: the largest attention blocks need the tail of Q.
            # First region is qq-major (paced by wq quarter arrivals); the
            # rest are oc-major so QT evacuations pipeline per-oc.
            for ts in (1, 0):
                ps = [pb.tile([_P, 512], f32, name=f"q{ts}_{oc}",
                              tag=f"b{oc}") for oc in range(8)]

                def q_mm(qh, qq, dcl, oc):
                    nc.tensor.matmul(
                        ps[oc][:, qh * 256:(qh + 1) * 256],
                        wq_q[qq][:, dcl, oc * _P:(oc + 1) * _P],
                        xq_t[2 * ts + qh][:, 2 * qq + dcl, :],
                        start=(qq == 0 and dcl == 0),
                        stop=(qq == 3 and dcl == 1))

                if ts == 1:
                    for qq in range(4):
                        for dcl in range(2):
                            for oc in range(8):
                                q_mm(1, qq, dcl, oc)
                else:
                    for oc in range(8):
                        for qq in range(4):
                            for dcl in range(2):
                                q_mm(1, qq, dcl, oc)
                for oc in range(8):
                    for qq in range(4):
                        for dcl in range(2):
                            q_mm(0, qq, dcl, oc)
                    dst = QT[:, oc, ts * 512:(ts + 1) * 512]
                    if oc % 2 == 0:
                        nc.scalar.copy(dst, ps[oc])
                    else:
                        nc.vector.tensor_copy(dst, ps[oc])

            # ---- Phase C: attention per query block, largest first.
            # Software-pipelined: block j's context is emitted after block
            # j-1's scores, so the PE fills the exp/transpose latency of
            # one block with the score matmuls of the next. ----
            def scores_block(j):
                km = 256 * (j + 1)
                nsl = (km + 511) // 512
                pexp = ppsb.tile([_P, _T], bf16, name=f"pexp{j}",
                                 tag="pexp")
                denoms = psm.tile([_P, 4], f32, name=f"dn{j}", tag="denoms")
                for ks in range(nsl):
                    w = min(512, km - ks * 512)
                    ps = pb.tile([_P, 512], f32, name=f"s{j}_{ks}",
                                 tag=f"b{ks % 2}")
                    for oc in range(8):
                        nc.tensor.matmul(
                            ps[:, :w],
                            QT[:, oc, j * _P:(j + 1) * _P],
                            KT[:, oc, ks * 512:ks * 512 + w],
                            start=(oc == 0), stop=(oc == 7))
                    if ks == nsl - 1:
                        nc.vector.tensor_add(
                            ps[:, w - 256:w], ps[:, w - 256:w], cmask_sb)
                    nc.scalar.activation(
                        out=pexp[:, ks * 512:ks * 512 + w], in_=ps[:, :w],
                        func=mybir.ActivationFunctionType.Exp,
                        bias=0.0, scale=_GAMMA,
                        accum_out=denoms[:, ks:ks + 1])
                return pexp, denoms

            def context_block(j, pexp, denoms):
                nkb = 2 * (j + 1)
                nsl = (256 * (j + 1) + 511) // 512
                denom = psm.tile([_P, 1], f32, name=f"d{j}", tag="denom")
                nc.vector.tensor_reduce(
                    out=denom, in_=denoms[:, :nsl],
                    axis=mybir.AxisListType.X, op=mybir.AluOpType.add)
                rden = psm.tile([_P, 1], f32, name=f"r{j}", tag="rden")
                nc.vector.reciprocal(rden, denom)
                cps = [pb.tile([_P, 512], f32, name=f"c{j}_{ns}",
                               tag=f"b{4 + 2 * (j % 2) + ns}")
                       for ns in range(2)]
                for kb in range(nkb):
                    pt_ps = pb.tile([_P, _P], bf16, name=f"pt{j}_{kb}",
                                    tag=f"b{2 + kb % 2}")
                    nc.tensor.transpose(
                        pt_ps, pexp[:, kb * _P:(kb + 1) * _P], identb)
                    pt_sb = ppt.tile([_P, _P], bf16, name=f"pts{j}_{kb}",
                                     tag="pts")
                    nc.vector.tensor_copy(pt_sb, pt_ps)
                    for ns in range(2):
                        nc.tensor.matmul(
                            cps[ns],
                            pt_sb,
                            V[:, kb, ns * 512:(ns + 1) * 512],
                            start=(kb == 0), stop=(kb == nkb - 1))
                ctx_sb = pctx.tile([_P, _D], f32, name=f"cs{j}",
                                   tag="ctxsb")
                for ns in range(2):
                    nc.vector.tensor_scalar_mul(
                        ctx_sb[:, ns * 512:(ns + 1) * 512], cps[ns], rden)
                nc.sync.dma_start(
                    out=out[j * _P:(j + 1) * _P, :], in_=ctx_sb)

            for j in reversed(range(_NQB)):
                pexp, denoms = scores_block(j)
                context_block(j, pexp, denoms)

    nc.finalize()
    return nc


def _qrows(h: int) -> np.ndarray:
    """Global query-row indices handled by half h, in core-local order."""
    blocks = np.arange(_NQB) * 2 + h          # global block ids, 8 of them
    return (blocks[:, None] * _P + np.arange(_P)[None, :]).reshape(-1)


def _host_inputs(x, w_query, w_key, w_value, mm_mode: str):
    if mm_mode == "bf16":
        import ml_dtypes
        cdt = ml_dtypes.bfloat16
    else:
        cdt = np.float32
    wqT = np.ascontiguousarray(np.asarray(w_query, np.float32).T).astype(cdt)
    wkT = np.ascontiguousarray(np.asarray(w_key, np.float32).T).astype(cdt)
    wvT = np.ascontiguousarray(np.asarray(w_value, np.float32).T).astype(cdt)
    x = np.asarray(x, np.float32)

    in_maps = []
    for c in range(8):
        b, h = c // 2, c % 2
        xb = x[b]                                     # [T, D]
        qr = _qrows(h)
        xT = np.ascontiguousarray(xb.T).astype(cdt)             # [D, T]
        xqT = np.ascontiguousarray(xb[qr].T).astype(cdt)        # [D, 1024]
        # cmask[p, c2] = 0 if c2 <= p + 128*h else -1e9
        p = np.arange(_P)[:, None]
        c2 = np.arange(2 * _P)[None, :]
        cmask = np.where(c2 <= p + _P * h, 0.0, _NEG).astype(np.float32)
        in_maps.append({
            "xT": xT, "xqT": xqT,
            "wqT": wqT, "wkT": wkT, "wvT": wvT,
            "cmask": cmask,
        })
    return in_maps


def _gather(results):
    out = np.empty((_B, _T, _D), np.float32)
    for c in range(8):
        b, h = c // 2, c % 2
        out[b, _qrows(h)] = results[c]["out"]
    return out


def kernel(x, w_query, w_key, w_value, _trace=False):
    key = (_MM_MODE, _SUB_MAX)
    if key not in _CACHE:
        _CACHE[key] = _build_nc(_MM_MODE, _SUB_MAX)
    nc = _CACHE[key]
    in_maps = _host_inputs(x, w_query, w_key, w_value, _MM_MODE)
    from concourse.bass_utils import run_bass_kernel_spmd
    res = run_bass_kernel_spmd(nc, in_maps, core_ids=list(range(8)),
                               trace=_trace)
    out = _gather(res.results)
    if _trace:
        return out, res
    return out


# revision 24
# speedup vs baseline: 2.4751x; 2.3257x over previous
"""Causal attention kernel for Trainium2 (Bass/Tile), 8-core SPMD.

Problem: x:(4,2048,1024), w_{q,k,v}:(1024,1024) fp32.
  q/k/v = x @ w.T ; scores = (q @ k.T)/sqrt(1024) causal-masked; out = softmax @ v.

Sharding: core c -> batch b=c//2, half h=c%2. Each batch's 16 query blocks
(128 rows) are interleaved even/odd between its two cores (core-local block
j <-> global block g=2j+h), so causal work is balanced. Q^T is projected for
the core's own 1024 query rows only; K^T and V are projected IN FULL on both
cores of a pair (redundant compute, ~55us extra PE time) — this removes the
pair AllGather of the earlier design, which serialized ~450us of collective
time and stalled every engine. Zero inter-core communication remains; the
program is SPMD (per-core difference is input DATA: xqT row choice and the
cmask diagonal offset encoding h).

Layouts (host pre-transposes, so no on-chip transposes of inputs needed):
  xT    [D, T]    = x[b].T        (d_in on partitions for K/V projections)
  xqT   [D, 1024] = x[b][qrows].T
  w*T   [D, D]    = w.T           ([d_in, d_out])
  cmask [128, 256] additive mask for the last two key blocks of each qblock
SBUF residency (per partition): KT fp32r 64KB + V bf16 32KB + QT fp32r 32KB;
V (and the softmax weights P) are bf16 so full K/V fit without collectives.
Phase A streams x in four [128,8,512] chunks (double-buffered): per chunk
K^T cols (dc-outer, 8 PSUM banks) then V rows (2-bank tiles, tb-serial),
PSUM evacuations alternate ACT/DVE so the PE never waits long on a single
evac chain. Phase B projects Q^T (ts=1 first so attention can start early).
Phase C per query block (largest first): scores into PSUM slices (PE) ->
mask add (DVE) -> exp+row-sum straight from PSUM (ACT, fused accum_out,
bf16 out) -> P^T via PE transpose -> context matmul (PE, bf16) -> scale by
1/rowsum (DVE) -> DMA out.
Matmul datapath: fp32r for projections/scores (fp32 storage, 1 cyc/row at
N>=256), bf16 for P@V. Measured output rel err vs fp32 reference ~6e-4.
"""

import numpy as np

_B, _T, _D = 4, 2048, 1024
_P = 128
_NQB = 8              # query blocks per core
_TQ = _NQB * _P       # 1024 query rows per core
_GAMMA = 1.0 / 32.0   # 1/sqrt(D)
_NEG = -1.0e9

_MM_MODE = "fp32r"    # projections/scores datapath
_SUB_MAX = False      # scores ~N(0,1): exp cannot overflow, skip row-max

_CACHE = {}


def _build_nc(mm_mode: str, sub_max: bool = False):
    import concourse.bass as bass  # noqa: F401
    import concourse.mybir as mybir
    import concourse.tile as tile
    from concourse import bacc
    from concourse.masks import make_identity
    from contextlib import ExitStack

    f32 = mybir.dt.float32
    bf16 = mybir.dt.bfloat16
    if mm_mode == "bf16":
        mdt = bf16
    elif mm_mode == "fp32r":
        mdt = mybir.dt.float32r
    else:
        mdt = f32

    nc = bacc.Bacc(None, target_bir_lowering=False)
    xT = nc.dram_tensor("xT", [_D, _T], mdt, kind="ExternalInput")
    xqT = nc.dram_tensor("xqT", [_D, _TQ], mdt, kind="ExternalInput")
    wqT = nc.dram_tensor("wqT", [_D, _D], mdt, kind="ExternalInput")
    wkT = nc.dram_tensor("wkT", [_D, _D], mdt, kind="ExternalInput")
    wvT = nc.dram_tensor("wvT", [_D, _D], mdt, kind="ExternalInput")
    cmask = nc.dram_tensor("cmask", [_P, 2 * _P], f32, kind="ExternalInput")
    out = nc.dram_tensor("out", [_TQ, _D], f32, kind="ExternalOutput")

    xT_v = xT.rearrange("(a p) t -> p a t", p=_P)       # [128, 8, 2048]
    xqT_v = xqT.rearrange("(a p) t -> p a t", p=_P)     # [128, 8, 1024]
    wqT_v = wqT.rearrange("(a p) o -> p a o", p=_P)
    wkT_v = wkT.rearrange("(a p) o -> p a o", p=_P)
    wvT_v = wvT.rearrange("(a p) o -> p a o", p=_P)

    _NCH = 4            # x chunks of 512 key-cols
    _CW = _T // _NCH    # 512

    with ExitStack() as ctx:
        tc = ctx.enter_context(tile.TileContext(nc))
        const = ctx.enter_context(tc.tile_pool(name="const", bufs=1))

        ident_f32 = const.tile([_P, _P], f32, tag="identf")
        make_identity(nc, ident_f32)
        identb = const.tile([_P, _P], bf16, tag="identb")
        nc.vector.tensor_copy(identb, ident_f32)
        cmask_sb = const.tile([_P, 2 * _P], f32, tag="cmask")
        nc.gpsimd.dma_start(out=cmask_sb, in_=cmask[:, :])

        # persistent SBUF residents (QT allocated after phase A pools close)
        pres = ctx.enter_context(tc.tile_pool(name="pres", bufs=1))
        KT = pres.tile([_P, 8, _T], mdt, tag="kt")          # [o-part, oc, t]
        V = pres.tile([_P, 16, _D], bf16, tag="v")          # [t-part, tt, o]
        # pre-allocated staging tiles: dodge the pool-release drain barrier
        # so the first xq quarter and first wq slice land during phase A
        pst = ctx.enter_context(tc.tile_pool(name="pst", bufs=1))
        xq3_st = pst.tile([_P, 8, 256], mdt, tag="st")
        wq0_st = pst.tile([_P, 1, _D], mdt, tag="stw")

        # ---- Phase A: K-pass then V-pass. x is streamed twice so wk+wv
        # are the only resident weights; DMA queue order matches
        # consumption order so the PE never waits mid-pass. ----
        with tc.tile_pool(name="pa_x", bufs=2) as px, \
             tc.tile_pool(name="pa_w", bufs=1) as pw, \
             tc.tile_pool(name="pa_ps", bufs=1, space="PSUM") as pp:

            def load_chunk(chi, nm, eng=None):
                ch = px.tile([_P, 8, _CW], mdt, name=nm, tag="xc")
                (eng or nc.sync).dma_start(
                    out=ch, in_=xT_v[:, :, chi * _CW:(chi + 1) * _CW])
                return ch

            wk_sb = pw.tile([_P, 8, _D], mdt, tag="wk")
            wv_sb = pw.tile([_P, 8, _D], mdt, tag="wv")
            # SP queue: wk/ch0 interleaved, ch1, ch2, wv, ch3, xq3 stage,
            # then (emitted later) wq, xq, out. V-pass x reloads go on the
            # idle Pool queue so phase B loads aren't stuck behind them.
            ch0 = px.tile([_P, 8, _CW], mdt, name="ch0", tag="xc")
            for dc in range(8):
                nc.sync.dma_start(out=wk_sb[:, dc, :], in_=wkT_v[:, dc, :])
                nc.sync.dma_start(out=ch0[:, dc, :], in_=xT_v[:, dc, 0:_CW])
            ch1 = load_chunk(1, "ch1")
            ch2 = load_chunk(2, "ch2")
            for dc in range(8):
                nc.sync.dma_start(out=wv_sb[:, dc, :], in_=wvT_v[:, dc, :])
            ch3 = load_chunk(3, "ch3")
            nc.sync.dma_start(out=xq3_st,
                              in_=xqT_v[:, :, 3 * 256:4 * 256])
            nc.sync.dma_start(out=wq0_st, in_=wqT_v[:, 0:1, :])

            # K-pass. ch0 is dc-outer (matches the interleaved DMA arrival
            # order); ch1-3 are oc-outer so each bank's evacuation pipelines
            # under the next oc's accumulation instead of bunching at the
            # chunk boundary (which would stall the PE and reset its p-state
            # ramp to 1.2GHz).
            def k_evac(chi, oc, t):
                dst = KT[:, oc, chi * _CW:(chi + 1) * _CW]
                if oc % 2 == 0:
                    nc.scalar.copy(dst, t)
                else:
                    nc.vector.tensor_copy(dst, t)

            ps = [pp.tile([_P, _CW], f32, name=f"k0_{oc}", tag=f"b{oc}")
                  for oc in range(8)]
            for dc in range(8):
                for oc in range(8):
                    nc.tensor.matmul(
                        ps[oc], wk_sb[:, dc, oc * _P:(oc + 1) * _P],
                        ch0[:, dc, :], start=(dc == 0), stop=(dc == 7))
            for oc in range(8):
                k_evac(0, oc, ps[oc])
            for chi, ch in ((1, ch1), (2, ch2), (3, ch3)):
                for oc in range(8):
                    t = pp.tile([_P, _CW], f32, name=f"k{chi}_{oc}",
                                tag=f"b{oc}")
                    for dc in range(8):
                        nc.tensor.matmul(
                            t, wk_sb[:, dc, oc * _P:(oc + 1) * _P],
                            ch[:, dc, :], start=(dc == 0), stop=(dc == 7))
                    k_evac(chi, oc, t)

            # V-pass (x streamed again, reloads on the Pool DMA queue)
            for chi in range(4):
                ch = load_chunk(chi, f"cv{chi}", eng=nc.gpsimd)
                for tb in range(4):
                    psv = [pp.tile([_P, 512], f32, name=f"v{chi}_{tb}_{ns}",
                                   tag=f"b{2 * tb + ns}") for ns in range(2)]
                    for dc in range(8):
                        for ns in range(2):
                            nc.tensor.matmul(
                                psv[ns],
                                ch[:, dc, tb * _P:(tb + 1) * _P],
                                wv_sb[:, dc, ns * 512:(ns + 1) * 512],
                                start=(dc == 0), stop=(dc == 7))
                    tt = chi * 4 + tb
                    nc.scalar.copy(V[:, tt, 0:512], psv[0])
                    nc.vector.tensor_copy(V[:, tt, 512:_D], psv[1])

        # ---- Phases B+C share one scope: QT lives through attention;
        # one 8-bank PSUM pool serves Q projection (tags b0..b7), then
        # scores (b0/b1), P^T (b2/b3) and context halves (b4..b7). ----
        with tc.tile_pool(name="pb_qt", bufs=1) as pqt, \
             tc.tile_pool(name="pb_w", bufs=1) as pwq, \
             tc.tile_pool(name="pb_x", bufs=2) as pxq, \
             tc.tile_pool(name="pc_p", bufs=2) as ppsb, \
             tc.tile_pool(name="pc_pt", bufs=3) as ppt, \
             tc.tile_pool(name="pc_ctx", bufs=2) as pctx, \
             tc.tile_pool(name="pc_small", bufs=4) as psm, \
             tc.tile_pool(name="pbc_ps", bufs=1, space="PSUM") as pb:
            QT = pqt.tile([_P, 8, _TQ], mdt, tag="qt")      # [o-part, oc, q]
            # wq streamed as eight per-dc slices [*,1,1024] so the Q
            # matmuls start as soon as each slice lands (the pool alloc
            # itself is gated on phase A's drain). Slice dc=0 and xq
            # quarter tsq=3 were pre-staged during phase A.
            wq_d = [wq0_st] + [pwq.tile([_P, 1, _D], mdt, name=f"wqd{dc}",
                                        tag=f"wq{dc}") for dc in range(1, 8)]
            xq_t = {3: xq3_st}
            for dc in (1, 2, 3):
                nc.sync.dma_start(out=wq_d[dc], in_=wqT_v[:, dc:dc + 1, :])
            for tsq in (2, 1, 0):
                xq_t[tsq] = pxq.tile([_P, 8, 256], mdt, name=f"xq{tsq}",
                                     tag="xq")
                if tsq == 2:
                    nc.sync.dma_start(
                        out=xq_t[2], in_=xqT_v[:, :, 2 * 256:3 * 256])
                    for dc in (4, 5, 6, 7):
                        nc.sync.dma_start(out=wq_d[dc],
                                          in_=wqT_v[:, dc:dc + 1, :])
                else:
                    nc.sync.dma_start(
                        out=xq_t[tsq],
                        in_=xqT_v[:, :, tsq * 256:(tsq + 1) * 256])
            # ts=1 first: the largest attention blocks need the tail of Q.
            # First region is dc-major (paced by wq slice arrivals); the
            # rest are oc-major so QT evacuations pipeline per-oc.
            for ts in (1, 0):
                ps = [pb.tile([_P, 512], f32, name=f"q{ts}_{oc}",
                              tag=f"b{oc}") for oc in range(8)]

                def q_mm(qh, dc, oc):
                    nc.tensor.matmul(
                        ps[oc][:, qh * 256:(qh + 1) * 256],
                        wq_d[dc][:, 0, oc * _P:(oc + 1) * _P],
                        xq_t[2 * ts + qh][:, dc, :],
                        start=(dc == 0), stop=(dc == 7))

                if ts == 1:
                    for dc in range(8):
                        for oc in range(8):
                            q_mm(1, dc, oc)
                else:
                    for oc in range(8):
                        for dc in range(8):
                            q_mm(1, dc, oc)
                for oc in range(8):
                    for dc in range(8):
                        q_mm(0, dc, oc)
                    dst = QT[:, oc, ts * 512:(ts + 1) * 512]
                    if oc % 2 == 0:
                        nc.scalar.copy(dst, ps[oc])
                    else:
                        nc.vector.tensor_copy(dst, ps[oc])

            # ---- Phase C: attention per query block, largest first.
            # Software-pipelined: block j's context is emitted after block
            # j-1's scores, so the PE fills the exp/transpose latency of
            # one block with the score matmuls of the next. ----
            def scores_block(j):
                km = 256 * (j + 1)
                nsl = (km + 511) // 512
                pexp = ppsb.tile([_P, _T], bf16, name=f"pexp{j}",
                                 tag="pexp")
                denoms = psm.tile([_P, 4], f32, name=f"dn{j}", tag="denoms")
                for ks in range(nsl):
                    w = min(512, km - ks * 512)
                    ps = pb.tile([_P, 512], f32, name=f"s{j}_{ks}",
                                 tag=f"b{(j + ks) % 2}")
                    for oc in range(8):
                        nc.tensor.matmul(
                            ps[:, :w],
                            QT[:, oc, j * _P:(j + 1) * _P],
                            KT[:, oc, ks * 512:ks * 512 + w],
                            start=(oc == 0), stop=(oc == 7))
                    if ks == nsl - 1:
                        nc.vector.tensor_add(
                            ps[:, w - 256:w], ps[:, w - 256:w], cmask_sb)
                    nc.scalar.activation(
                        out=pexp[:, ks * 512:ks * 512 + w], in_=ps[:, :w],
                        func=mybir.ActivationFunctionType.Exp,
                        bias=0.0, scale=_GAMMA,
                        accum_out=denoms[:, ks:ks + 1])
                return pexp, denoms

            def context_block(j, pexp, denoms):
                nkb = 2 * (j + 1)
                nsl = (256 * (j + 1) + 511) // 512
                denom = psm.tile([_P, 1], f32, name=f"d{j}", tag="denom")
                nc.vector.tensor_reduce(
                    out=denom, in_=denoms[:, :nsl],
                    axis=mybir.AxisListType.X, op=mybir.AluOpType.add)
                rden = psm.tile([_P, 1], f32, name=f"r{j}", tag="rden")
                nc.vector.reciprocal(rden, denom)
                cps = [pb.tile([_P, 512], f32, name=f"c{j}_{ns}",
                               tag=f"b{4 + 2 * (j % 2) + ns}")
                       for ns in range(2)]
                for kb in range(nkb):
                    pt_ps = pb.tile([_P, _P], bf16, name=f"pt{j}_{kb}",
                                    tag=f"b{2 + kb % 2}")
                    nc.tensor.transpose(
                        pt_ps, pexp[:, kb * _P:(kb + 1) * _P], identb)
                    pt_sb = ppt.tile([_P, _P], bf16, name=f"pts{j}_{kb}",
                                     tag="pts")
                    nc.vector.tensor_copy(pt_sb, pt_ps)
                    for ns in range(2):
                        nc.tensor.matmul(
                            cps[ns],
                            pt_sb,
                            V[:, kb, ns * 512:(ns + 1) * 512],
                            start=(kb == 0), stop=(kb == nkb - 1))
                ctx_sb = pctx.tile([_P, _D], f32, name=f"cs{j}",
                                   tag="ctxsb")
                for ns in range(2):
                    nc.vector.tensor_scalar_mul(
                        ctx_sb[:, ns * 512:(ns + 1) * 512], cps[ns], rden)
                nc.sync.dma_start(
                    out=out[j * _P:(j + 1) * _P, :], in_=ctx_sb)

            # tiny tail blocks (j=1,0) have no work to hide the exp
            # latency behind, so emit both scores before either context
            for j in reversed(range(2, _NQB)):
                pexp, denoms = scores_block(j)
                context_block(j, pexp, denoms)
            s1 = scores_block(1)
            s0 = scores_block(0)
            context_block(1, *s1)
            context_block(0, *s0)

    nc.finalize()
    return nc


def _qrows(h: int) -> np.ndarray:
    """Global query-row indices handled by half h, in core-local order."""
    blocks = np.arange(_NQB) * 2 + h          # global block ids, 8 of them
    return (blocks[:, None] * _P + np.arange(_P)[None, :]).reshape(-1)


def _host_inputs(x, w_query, w_key, w_value, mm_mode: str):
    if mm_mode == "bf16":
        import ml_dtypes
        cdt = ml_dtypes.bfloat16
    else:
        cdt = np.float32
    wqT = np.ascontiguousarray(np.asarray(w_query, np.float32).T).astype(cdt)
    wkT = np.ascontiguousarray(np.asarray(w_key, np.float32).T).astype(cdt)
    wvT = np.ascontiguousarray(np.asarray(w_value, np.float32).T).astype(cdt)
    x = np.asarray(x, np.float32)

    in_maps = []
    for c in range(8):
        b, h = c // 2, c % 2
        xb = x[b]                                     # [T, D]
        qr = _qrows(h)
        xT = np.ascontiguousarray(xb.T).astype(cdt)             # [D, T]
        xqT = np.ascontiguousarray(xb[qr].T).astype(cdt)        # [D, 1024]
        # cmask[p, c2] = 0 if c2 <= p + 128*h else -1e9
        p = np.arange(_P)[:, None]
        c2 = np.arange(2 * _P)[None, :]
        cmask = np.where(c2 <= p + _P * h, 0.0, _NEG).astype(np.float32)
        in_maps.append({
            "xT": xT, "xqT": xqT,
            "wqT": wqT, "wkT": wkT, "wvT": wvT,
            "cmask": cmask,
        })
    return in_maps


def _gather(results):
    out = np.empty((_B, _T, _D), np.float32)
    for c in range(8):
        b, h = c // 2, c % 2
        out[b, _qrows(h)] = results[c]["out"]
    return out


def kernel(x, w_query, w_key, w_value, _trace=False):
    key = (_MM_MODE, _SUB_MAX)
    if key not in _CACHE:
        _CACHE[key] = _build_nc(_MM_MODE, _SUB_MAX)
    nc = _CACHE[key]
    in_maps = _host_inputs(x, w_query, w_key, w_value, _MM_MODE)
    from concourse.bass_utils import run_bass_kernel_spmd
    res = run_bass_kernel_spmd(nc, in_maps, core_ids=list(range(8)),
                               trace=_trace)
    out = _gather(res.results)
    if _trace:
        return out, res
    return out
